# revision 10
# baseline (speedup 1.0000x reference)
"""Multi-head causal attention (B=2,S=2048,E=1024,H=16,D=64) on 8 NeuronCores.

Sharding: core c handles batch b=c//4 and head-group hg=c%4 (4 heads = 256
channels each).  Each core computes Q^T/K^T/V projections for its channel
slice, causal softmax attention for its 4 heads, and a partial output
projection through its slice of Wo.  Host sums the 4 partials per batch and
adds the bias.

v2.1: seq-blocked x DMA for an early start; K^T kept in head-pair-stacked
[128,S] tiles (single PSUM cast, no padding -- the other head's rows meet
zeros in Q^T); Q^T in per-head zero-padded tiles so every score matmul is a
uniform 128-partition config (64-partition matmuls measured ~1.5x slower and
poison neighbors with config switches); pair-fused softmax exp (one ACT
instruction per two key chunks, second chunk's matmul extended to the pair
window so exp never reads stale PSUM); PSUM->SBUF output copies on the Scalar
engine; bf16 output partials summed on host in fp32.
"""

import sys

sys.path.insert(0, "/opt/trn_rl_repo")

import numpy as np

B, S, E, H, D = 2, 2048, 1024, 16, 64
N_CORES = 8
HPC = 4               # heads per core
CH = HPC * D          # 256 channels per core
SBK = 512             # seq block (moving free dim)
NSB = S // SBK        # 4
NE = E // 128         # 8 contraction chunks
NKC = S // 128        # 16 key chunks

_BUILT = {}


def _build():
    if "nc" in _BUILT:
        return _BUILT["nc"]

    from contextlib import ExitStack

    import concourse.bacc as bacc
    import concourse.tile as tile
    from concourse import mybir

    F32 = mybir.dt.float32
    BF16 = mybir.dt.bfloat16
    AF = mybir.ActivationFunctionType

    nc = bacc.Bacc("TRN2", target_bir_lowering=False, debug=False,
                   num_devices=N_CORES)
    xt = nc.dram_tensor("xt", [E, S], BF16, kind="ExternalInput").ap()
    wq = nc.dram_tensor("wq", [E, CH], BF16, kind="ExternalInput").ap()
    wk = nc.dram_tensor("wk", [E, CH], BF16, kind="ExternalInput").ap()
    wv = nc.dram_tensor("wv", [E, CH], BF16, kind="ExternalInput").ap()
    wo = nc.dram_tensor("wo", [CH, E], BF16, kind="ExternalInput").ap()
    tri = nc.dram_tensor("tri", [128, 128], BF16, kind="ExternalInput").ap()
    pout = nc.dram_tensor("pout", [S, E], BF16, kind="ExternalOutput").ap()

    with tile.TileContext(nc) as tc, ExitStack() as ctx:
        xtp = ctx.enter_context(tc.tile_pool(name="xtp", bufs=NE))
        wp = ctx.enter_context(tc.tile_pool(name="wp", bufs=3 * NE))
        wop = ctx.enter_context(tc.tile_pool(name="wop", bufs=2))
        qp = ctx.enter_context(tc.tile_pool(name="qp", bufs=4))
        kp = ctx.enter_context(tc.tile_pool(name="kp", bufs=2))
        vp = ctx.enter_context(tc.tile_pool(name="vp", bufs=NKC))
        trip = ctx.enter_context(tc.tile_pool(name="trip", bufs=1))
        onp = ctx.enter_context(tc.tile_pool(name="onp", bufs=2))
        ptp = ctx.enter_context(tc.tile_pool(name="ptp", bufs=6))
        recp = ctx.enter_context(tc.tile_pool(name="recp", bufs=2))
        bcp = ctx.enter_context(tc.tile_pool(name="bcp", bufs=2))
        oop = ctx.enter_context(tc.tile_pool(name="oop", bufs=3))
        ohp = ctx.enter_context(tc.tile_pool(name="ohp", bufs=8))
        pp = ctx.enter_context(tc.tile_pool(name="pp", bufs=2, space="PSUM"))
        sp = ctx.enter_context(tc.tile_pool(name="sp", bufs=2, space="PSUM"))
        avp = ctx.enter_context(tc.tile_pool(name="avp", bufs=2, space="PSUM"))

        # --- x^T loads: sb0 chunk-by-chunk for an early start, rest fused ---
        xts = [xtp.tile([128, S], BF16, tag="xt", name=f"xt{e}")
               for e in range(NE)]
        for e in range(NE):
            nc.sync.dma_start(xts[e][:, 0:SBK], xt[e * 128:(e + 1) * 128,
                                                   0:SBK])
        for e in range(NE):
            nc.sync.dma_start(xts[e][:, SBK:S],
                              xt[e * 128:(e + 1) * 128, SBK:S])
        tri_sb = trip.tile([128, 128], BF16, tag="tri")
        nc.sync.dma_start(tri_sb[:], tri[:, :])
        wqs, wks, wvs = [], [], []
        for lst, srcw, tg in ((wqs, wq, "wq"), (wks, wk, "wk"),
                              (wvs, wv, "wv")):
            for e in range(NE):
                t = wp.tile([128, CH], BF16, tag=tg)
                nc.gpsimd.dma_start(t[:], srcw[e * 128:(e + 1) * 128, :])
                lst.append(t)
        wos = []
        for cc in range(2):
            t = wop.tile([128, E], BF16, tag="wo")
            nc.gpsimd.dma_start(t[:], wo[cc * 128:(cc + 1) * 128, :])
            wos.append(t)

        ones_sb = trip.tile([128, HPC], BF16, tag="ones")
        nc.vector.memset(ones_sb[:], 1.0)
        wrm = trip.tile([128, 128], BF16, tag="wrm")
        nc.vector.memset(wrm[:], 0.125)

        # kt: per head-pair cc, [128, S]; head 2cc in partitions 0:64,
        # head 2cc+1 in 64:128 (the projection PSUM layout, single cast).
        # qt: per head, zero-padded so score matmuls contract 128 partitions.
        kts = [kp.tile([128, S], BF16, tag="kt", name=f"kt{cc}")
               for cc in range(2)]
        qts = [qp.tile([128, S], BF16, tag="qt", name=f"qt{h}")
               for h in range(HPC)]
        for h in range(HPC):
            z0 = 64 if h % 2 == 0 else 0  # zero the unused half
            nc.vector.memset(qts[h][z0:z0 + 64, :], 0.0)
        vts = [vp.tile([128, HPC * 65], BF16, tag="v", name=f"v{i}")
               for i in range(NKC)]
        for t in vts:
            nc.vector.tensor_copy(
                t[:].rearrange("p (h c) -> p h c", h=HPC)[:, :, 64:65],
                ones_sb[:].unsqueeze(2))
        ons = [onp.tile([128, S], BF16, tag="on", name=f"on{i}")
               for i in range(2)]

        # ---- dense-matmul group emitters ----
        def q_group(cc, sb):
            ps = pp.tile([128, SBK], F32, tag="pp", name=f"ps_q{cc}{sb}")
            for e in range(NE):
                nc.tensor.matmul(
                    ps[:], lhsT=wqs[e][:, cc * 128:(cc + 1) * 128],
                    rhs=xts[e][:, sb * SBK:(sb + 1) * SBK],
                    start=(e == 0), stop=(e == NE - 1))
            cols = slice(sb * SBK, (sb + 1) * SBK)
            nc.vector.tensor_copy(qts[2 * cc][0:64, cols], ps[0:64, :])
            nc.vector.tensor_copy(qts[2 * cc + 1][64:128, cols],
                                  ps[64:128, :])

        def k_group(cc, sb):
            ps = pp.tile([128, SBK], F32, tag="pp", name=f"ps_k{cc}{sb}")
            for e in range(NE):
                nc.tensor.matmul(
                    ps[:], lhsT=wks[e][:, cc * 128:(cc + 1) * 128],
                    rhs=xts[e][:, sb * SBK:(sb + 1) * SBK],
                    start=(e == 0), stop=(e == NE - 1))
            nc.vector.tensor_copy(kts[cc][:, sb * SBK:(sb + 1) * SBK], ps[:])

        def v_group(sc):
            ps = pp.tile([128, SBK], F32, tag="pp", name=f"ps_v{sc}")
            for e in range(NE):
                nc.tensor.matmul(ps[:, 0:CH],
                                 lhsT=xts[e][:, sc * 128:(sc + 1) * 128],
                                 rhs=wvs[e][:], start=(e == 0),
                                 stop=(e == NE - 1))
            dst = vts[sc][:].rearrange("p (h c) -> p h c", h=HPC)
            nc.vector.tensor_copy(
                dst[:, :, 0:64],
                ps[:, 0:CH].rearrange("p (h c) -> p h c", h=HPC))

        oh = {}

        def wo_half(sc, eb):
            ps = pp.tile([128, SBK], F32, tag="pp", name=f"ph_o{sc}{eb}")
            nc.tensor.matmul(ps[:], lhsT=ons[0][:, sc * 128:(sc + 1) * 128],
                             rhs=wos[0][:, eb * SBK:(eb + 1) * SBK],
                             start=True, stop=True)
            t = ohp.tile([128, SBK], F32, tag="oh", name=f"oh{sc}{eb}")
            nc.scalar.copy(t[:], ps[:])
            oh[(sc, eb)] = t

        def wo_combine(sc, eb):
            ps = pp.tile([128, SBK], F32, tag="pp", name=f"pc_o{sc}{eb}")
            nc.tensor.matmul(ps[:], lhsT=ons[1][:, sc * 128:(sc + 1) * 128],
                             rhs=wos[1][:, eb * SBK:(eb + 1) * SBK],
                             start=True, stop=True)
            oo = oop.tile([128, SBK], BF16, tag="oo", name=f"oc{sc}{eb}")
            nc.vector.tensor_add(oo[:], ps[:], oh[(sc, eb)][:])
            eng = nc.sync if (sc + eb) % 2 == 0 else nc.gpsimd
            eng.dma_start(
                pout[sc * 128:(sc + 1) * 128, eb * SBK:(eb + 1) * SBK],
                oo[:])

        def wo_group(sc, eb):
            ps = pp.tile([128, SBK], F32, tag="pp", name=f"ps_o{sc}{eb}")
            for cc in range(2):
                nc.tensor.matmul(ps[:],
                                 lhsT=ons[cc][:, sc * 128:(sc + 1) * 128],
                                 rhs=wos[cc][:, eb * SBK:(eb + 1) * SBK],
                                 start=(cc == 0), stop=(cc == 1))
            oo = oop.tile([128, SBK], BF16, tag="oo", name=f"oo{sc}{eb}")
            nc.scalar.copy(oo[:], ps[:])
            nc.sync.dma_start(
                pout[sc * 128:(sc + 1) * 128, eb * SBK:(eb + 1) * SBK],
                oo[:])

        def warm_mm(n):
            for _ in range(n):
                wps = pp.tile([128, SBK], F32, tag="pp", name="warmps")
                nc.tensor.matmul(wps[:, 0:128], lhsT=wrm[:], rhs=wrm[:],
                                 start=True, stop=True, skip_group_check=True)

        from collections import deque
        fillers = deque()
        warm_mm(30)

        # prologue: only what attention (qb0, h0/h1) needs
        q_group(0, 0)
        k_group(0, 0)
        for sc in range(4):
            v_group(sc)
        fillers.append(lambda: q_group(1, 0))
        fillers.append(lambda: k_group(1, 0))
        for sb in range(1, 3):
            fillers.append(lambda sb=sb: q_group(0, sb))
            fillers.append(lambda sb=sb: k_group(0, sb))
            fillers.append(lambda sb=sb: q_group(1, sb))
            fillers.append(lambda sb=sb: k_group(1, sb))
        for sc in range(4, 8):
            fillers.append(lambda sc=sc: v_group(sc))
        fillers.append(lambda: q_group(0, 3))
        fillers.append(lambda: k_group(0, 3))
        fillers.append(lambda: q_group(1, 3))
        fillers.append(lambda: k_group(1, 3))

        # ---- attention: qb-outer; pair-fused exp ----
        for qb in range(NSB):
            if qb >= 1:
                for sc in range(4 * qb, 4 * (qb + 1)):
                    if sc + 4 < NKC:
                        fillers.append(lambda sc=sc: v_group(sc + 4))
                for sc in range(4 * (qb - 1), 4 * qb):
                    for eb in range(2):
                        fillers.append(
                            lambda sc=sc, eb=eb: wo_group(sc, eb))
            for h in range(HPC):
                if qb == NSB - 1 and h == 2:
                    # h0/h1 of the last q-block are done: their half of the
                    # final output projection can overlap h2/h3
                    for sc in range(12, NKC):
                        for eb in range(2):
                            fillers.append(
                                lambda sc=sc, eb=eb: wo_half(sc, eb))
                cc, hf = h // 2, h % 2
                qt, kt = qts[h], kts[cc]
                av = avp.tile([65, SBK], F32, tag="av", name=f"av{qb}{h}")
                nk = 4 * (qb + 1)
                pend = deque()  # AV lags scores; drain between score pairs
                def flush_av(nmax, final=False):
                    n = 0
                    while pend and n < nmax:
                        pkc, pj0, ppt, psub = pend.popleft()
                        nc.tensor.matmul(
                            av[:, pj0:SBK],
                            lhsT=vts[pkc][:, h * 65:(h + 1) * 65],
                            rhs=ppt[:, psub, pj0:SBK],
                            start=(pkc == 0),
                            stop=(final and not pend),
                            skip_group_check=True)
                        n += 1
                for kc2 in range(0, nk, 2):
                    ss = sp.tile([128, 2, SBK], F32, tag="sp")
                    j0m = max(0, kc2 * 128 - qb * SBK)
                    for i, kc in enumerate((kc2, kc2 + 1)):
                        k0 = kc * 128
                        j0 = max(0, k0 - qb * SBK)
                        # write from j0m so the pair-fused exp below never
                        # reads stale PSUM (AV still reads from j0)
                        nc.tensor.matmul(
                            ss[:, i, j0m:SBK],
                            lhsT=kt[:, k0:k0 + 128],
                            rhs=qt[:, qb * SBK + j0m:(qb + 1) * SBK],
                            start=True, stop=True)
                        if fillers:
                            fillers.popleft()()
                        pend.append((kc, j0, None, i))
                    pt = ptp.tile([128, 2, SBK], BF16, tag="pt")
                    nc.scalar.activation(pt[:, :, j0m:SBK],
                                         ss[:, :, j0m:SBK],
                                         AF.Exp, scale=float(D) ** -0.5)
                    for n_ in range(2):
                        kc, j0, _, i = pend[-2 + n_]
                        pend[-2 + n_] = (kc, j0, pt, i)
                        if kc * 128 >= qb * SBK:  # diag chunk: mask band
                            nc.vector.tensor_mul(pt[:, i, j0:j0 + 128],
                                                 pt[:, i, j0:j0 + 128],
                                                 tri_sb[:])
                    if len(pend) >= 4:
                        flush_av(2)
                flush_av(99, final=True)
                rec = recp.tile([1, SBK], F32, tag="rec")
                nc.vector.tensor_copy(rec[:], av[64:65, :])
                rec2 = recp.tile([1, SBK], F32, tag="rec2")
                nc.vector.reciprocal_approx_fast(rec2[:], rec[:])
                bc = bcp.tile([64, SBK], F32, tag="bc")
                nc.gpsimd.partition_broadcast(bc[:], rec2[:])
                nc.vector.tensor_mul(
                    ons[cc][hf * 64:hf * 64 + 64,
                            qb * SBK:(qb + 1) * SBK],
                    av[0:64, :], bc[:])

        while fillers:
            fillers.popleft()()
        # tail: combine the remaining half of the last q-block's projection
        for sc in range(12, NKC):
            for eb in range(2):
                wo_combine(sc, eb)

    nc.compile()
    _BUILT["nc"] = nc
    return nc


def _install_ntff_shim():
    """Provide antenv.axon_hooks (missing in this image) so trace=True works."""
    import types
    try:
        from antenv.axon_hooks import get_axon_ntff_profile_hook  # noqa: F401
        return
    except ImportError:
        pass
    import antenv
    from trn_agent_boot.trn_boot import _ntff_profile_via_ctypes
    hook = _ntff_profile_via_ctypes("/opt/axon/libaxon_pjrt.so")
    mod = types.ModuleType("antenv.axon_hooks")
    mod._hook = hook
    mod.get_axon_ntff_profile_hook = lambda: mod._hook
    mod.set_axon_ntff_profile_hook = lambda h: setattr(mod, "_hook", h)
    sys.modules["antenv.axon_hooks"] = mod
    antenv.axon_hooks = mod


def kernel(x, Wq, Wk, Wv, Wo, bo, _trace=False):
    from concourse.bass_utils import run_bass_kernel_spmd

    nc = _build()

    x = np.asarray(x, dtype=np.float32)
    Wq = np.asarray(Wq, dtype=np.float32)
    Wk = np.asarray(Wk, dtype=np.float32)
    Wv = np.asarray(Wv, dtype=np.float32)
    Wo = np.asarray(Wo, dtype=np.float32)
    bo = np.asarray(bo, dtype=np.float32)

    import ml_dtypes
    bf = ml_dtypes.bfloat16
    tri = np.triu(np.ones((128, 128), dtype=np.float32)).astype(bf)
    xt_b = [np.ascontiguousarray(x[b].T) for b in range(B)]
    in_maps = []
    for c in range(N_CORES):
        b, hg = c // HPC, c % HPC
        sl = slice(hg * CH, (hg + 1) * CH)
        in_maps.append({
            "xt": xt_b[b].astype(bf),
            "wq": np.ascontiguousarray(Wq[:, sl]).astype(bf),
            "wk": np.ascontiguousarray(Wk[:, sl]).astype(bf),
            "wv": np.ascontiguousarray(Wv[:, sl]).astype(bf),
            "wo": np.ascontiguousarray(Wo[sl, :]).astype(bf),
            "tri": tri,
        })

    kwargs = {}
    if _trace:
        _install_ntff_shim()
        kwargs = dict(trace=True, trace_cores=[0])
    res = run_bass_kernel_spmd(nc, in_maps, core_ids=list(range(N_CORES)),
                               **kwargs)

    out = np.zeros((B, S, E), dtype=np.float32)
    for c in range(N_CORES):
        out[c // HPC] += res.results[c]["pout"].astype(np.float32)
    out += bo
    if _trace:
        return out, res
    return out
